# revision 24
# baseline (speedup 1.0000x reference)
"""LoRA LayerNorm Trainium2 kernel (8-core data-parallel, raw Bass).

out = x_hat * scale + shift, where
  x_hat    = (x - mean) * rsqrt(var + eps)        (LayerNorm over last dim)
  scale[i] = sum_r A_s[i,r] * B_s[r,i] * 2.0      (low-rank diagonal)
  shift[i] = sum_r A_h[i,r] * B_h[r,i] * 2.0

Sharding: x [2,4096,8192] -> 8192 rows, 1024 rows per core. LoRA params
replicated; each core computes scale/shift redundantly on device.

Per-core algorithm (rows on partitions, 8 tiles of [128, 8192]):
  setup: scale/shift diagonals via strided loads + DVE mul/reduce.
         scale broadcast [128, N] f32 built on-chip: K=2 PE matmul
         (ones2 x [hi;lo]) into PSUM + ACT copy to SBUF, where hi/lo is
         a two-term bf16 split of scale (error ~2^-17, not 2^-8). The
         start=True matmuls double as the PSUM has_written pre-set.
  per tile (8 psum banks as 4 chunk buffers of [128,1024]):
    DVE : bn_stats/bn_aggr -> mean,var; psum = (x - mean) * scale_bc
    ACT : std = sqrt(var+eps); out_sbuf = psum * rstd  (PSUM->SBUF)
    PE  : psum += std (x) shift  (K=1 bf16 rank-1, start=False)
    SP  : x tile loads (HWDGE, 2 MiB halves, 3 tile buffers)
    ACT : output stores (HWDGE ring)
    POOL: tiny SBUF->SBUF cast DMA std [128,1] f32 -> stdT [1,128] bf16

Same-engine RAW hazards (e.g. bn_aggr writing mv then the next DVE op
reading it) are NOT interlocked by the engines: they must be ordered
by waiting on the producer's own completion semaphore (cheap) -- raw
drains cost ~1.7us each and are avoided.
"""

import numpy as np
from contextlib import ExitStack

import concourse.bass as bass
from concourse import mybir
from concourse.bass_utils import run_bass_kernel_spmd

F32 = mybir.dt.float32
BF16 = mybir.dt.bfloat16

# Problem geometry (hardcoded; see module docstring)
B_DIM, S_DIM, N = 2, 4096, 8192
RANK = 4
SCALING = 2.0  # alpha / rank = 8 / 4
EPS = 1e-5
NCORES = 8
ROWS = B_DIM * S_DIM // NCORES  # 1024 rows per core
P = 128
NTILES = ROWS // P              # 8
NXB = 3                         # x tile buffers
CHUNK = 1024                    # psum chunk (2 banks)
NCHUNK = N // CHUNK             # 8
NPZ = 4                         # psum chunk buffers (8 banks total)
NSL = CHUNK // 512              # matmul slices per chunk (2)
HALF = N // 2                   # load granularity
QTR = N // 4                    # store granularity (4 outb buffers)
BN_F = 512                      # bn_stats max free dim
NBN = N // BN_F                 # 16
C = N // P                      # 64
# broadcast round order: second-fill rounds first so tile-0 stt chunks
# can start as soon as their pz buffer has had both fills copied out
BC_ORDER = [4, 5, 6, 7, 0, 1, 2, 3]


def build_nc() -> bass.Bass:
    nc = bass.Bass()

    x = nc.declare_dram_parameter("x_shard", [ROWS, N], F32, isOutput=False)
    sa = nc.declare_dram_parameter("lora_scale_A", [N, RANK], F32, isOutput=False)
    sb = nc.declare_dram_parameter("lora_scale_B", [RANK, N], F32, isOutput=False)
    ha = nc.declare_dram_parameter("lora_shift_A", [N, RANK], F32, isOutput=False)
    hb = nc.declare_dram_parameter("lora_shift_B", [RANK, N], F32, isOutput=False)
    y = nc.declare_dram_parameter("y_shard", [ROWS, N], F32, isOutput=True)

    with ExitStack() as ctx:
        ec = ctx.enter_context
        # big tiles
        xb = [ec(nc.sbuf_tensor(f"xb{i}", [P, N], F32)) for i in range(NXB)]
        outb = [ec(nc.sbuf_tensor(f"outb{i}", [P, QTR], F32)) for i in range(4)]
        scale_bc = ec(nc.sbuf_tensor("scale_bc", [P, N], F32))
        hilo_row = ec(nc.sbuf_tensor("hilo_row", [2, N], BF16))
        sh_row = ec(nc.sbuf_tensor("sh_row", [1, N], BF16))
        ones2 = ec(nc.sbuf_tensor("ones2", [2, P], BF16))
        # setup scratch (scale pair and shift pair loaded in parallel)
        a_t = ec(nc.sbuf_tensor("a_t", [P, C * RANK], F32))   # [128, 256]
        b_t = ec(nc.sbuf_tensor("b_t", [P, RANK * C], F32))
        a2_t = ec(nc.sbuf_tensor("a2_t", [P, C * RANK], F32))
        b2_t = ec(nc.sbuf_tensor("b2_t", [P, RANK * C], F32))
        prod = ec(nc.sbuf_tensor("prod", [P, C * RANK], F32))
        s_small = ec(nc.sbuf_tensor("s_small", [P, C], F32))  # [128, 64]
        t_small = ec(nc.sbuf_tensor("t_small", [P, C], F32))
        hi_small = ec(nc.sbuf_tensor("hi_small", [P, C], BF16))
        lo_small = ec(nc.sbuf_tensor("lo_small", [P, C], F32))
        # per-tile stats
        stats = ec(nc.sbuf_tensor("stats", [P, NBN * 6], F32))
        mv = [ec(nc.sbuf_tensor(f"mv{i}", [P, 2], F32)) for i in range(2)]
        stdb = [ec(nc.sbuf_tensor(f"stdb{i}", [P, 1], F32)) for i in range(4)]
        rstdb = [ec(nc.sbuf_tensor(f"rstdb{i}", [P, 1], F32)) for i in range(4)]
        stdT = [ec(nc.sbuf_tensor(f"stdT{i}", [1, P], BF16)) for i in range(4)]
        eps_t = ec(nc.sbuf_tensor("eps_t", [P, 1], F32))
        # psum: 4 chunk buffers x 2 banks
        pz = [ec(nc.psum_tensor(f"pz{i}", [P, CHUNK], F32)) for i in range(NPZ)]

        sems = {}
        load_names = [f"ld{q}{b}" for q in range(4) for b in range(NXB)]
        for s in (*load_names,
                  "store0", "store1", "store2", "store3",
                  "stdT0", "stdT1", "stdT2", "stdT3", "stt", "stats", "std", "rstd", "acc",
                  "copy", "const", "sdmaA", "sdmaB", "sdmaC", "sdmaD",
                  "dset", "gset", "pbc", "bset",
                  "vord", "bnend"):
            sems[s] = ec(nc.semaphore(s))
        # one sem per load-quarter DMA: DMAs sharing a sem can
        # interleave their 16 per-engine incs, so a mid-threshold does
        # NOT imply the first DMA completed.
        loadS = [[sems[f"ld{q}{b}"] for b in range(NXB)] for q in range(4)]
        storeS = [sems[f"store{i}"] for i in range(4)]
        stdTS = [sems[f"stdT{i}"] for i in range(4)]

        with nc.Block() as block:

            @block.sync
            def _(sp):
                # LoRA params first: tiny, HWDGE (fast), unblocks the
                # diag -> broadcast chain ~10us earlier than SWDGE
                sp.dma_start(
                    out=a_t[:],
                    in_=sa[:, :].rearrange("(p c) r -> p (c r)", p=P),
                ).then_inc(sems["sdmaA"], 16)
                sp.dma_start(
                    out=b_t[:].rearrange("p (r c) -> p r c", r=RANK),
                    in_=sb[:, :].rearrange("r (p c) -> p r c", p=P),
                ).then_inc(sems["sdmaB"], 16)
                sp.dma_start(
                    out=a2_t[:],
                    in_=ha[:, :].rearrange("(p c) r -> p (c r)", p=P),
                ).then_inc(sems["sdmaC"], 16)
                sp.dma_start(
                    out=b2_t[:].rearrange("p (r c) -> p r c", r=RANK),
                    in_=hb[:, :].rearrange("r (p c) -> p r c", p=P),
                ).then_inc(sems["sdmaD"], 16)
                for t in range(NTILES):
                    for qf in range(4):
                        if t >= NXB:
                            # xb quarter free once DVE finished the stt
                            # chunks of tile t-NXB that read it
                            sp.wait_ge(
                                sems["stt"],
                                NCHUNK * (t - NXB) + 2 * (qf + 1),
                            )
                        sp.dma_start(
                            out=xb[t % NXB][:, qf * QTR:(qf + 1) * QTR],
                            in_=x[t * P:(t + 1) * P, qf * QTR:(qf + 1) * QTR],
                        ).then_inc(loadS[qf][t % NXB], 16)

            @block.gpsimd
            def _(gp):
                gp.wait_ge(sems["dset"], 1)
                # hi/lo bf16 split of scale -> [2, N] bf16 row pair
                gp.dma_start(out=hilo_row[0:1, :], in_=hi_small[:]).then_inc(
                    sems["gset"], 16
                )
                gp.dma_start(out=hilo_row[1:2, :], in_=lo_small[:]).then_inc(
                    sems["gset"], 16
                )
                gp.wait_ge(sems["dset"], 2)
                # shift diag [128,64] f32 -> [1,N] bf16 row (cast DMA)
                gp.dma_start(out=sh_row[:], in_=t_small[:]).then_inc(
                    sems["gset"], 16
                )
                # per-tile: std [128,1] f32 -> stdT [1,128] bf16 (cast)
                for t in range(NTILES):
                    gp.wait_ge(sems["std"], t + 1)
                    if t >= 4:
                        # PE done reading stdT[t%4] (accums of tile t-4)
                        gp.wait_ge(sems["acc"], NCHUNK * (t - 3))
                    gp.dma_start(
                        out=stdT[t % 4][:], in_=stdb[t % 4][:]
                    ).then_inc(stdTS[t % 4], 16)

            @block.vector
            def _(v):
                v.memset(eps_t[:], EPS).then_inc(sems["const"], 1)
                v.memset(ones2[:], 1.0).then_inc(sems["const"], 1)
                # scale diag + hi/lo split.  vord orders same-engine RAW
                # (an op's reads may start before the previous op's writes
                # commit; waiting on the producer's completion sem is the
                # cheap alternative to a full pipe drain).
                v.wait_ge(sems["sdmaA"], 16)
                v.wait_ge(sems["sdmaB"], 16)
                v.tensor_mul(
                    prod[:].rearrange("p (c r) -> p c r", c=C),
                    a_t[:].rearrange("p (c r) -> p c r", c=C),
                    b_t[:].rearrange("p (r c) -> p c r", r=RANK),
                ).then_inc(sems["vord"], 1)
                v.wait_ge(sems["vord"], 1)
                v.tensor_reduce(
                    out=s_small[:].rearrange("p (c u) -> p c u", u=1),
                    in_=prod[:].rearrange("p (c r) -> p c r", c=C),
                    axis=mybir.AxisListType.X,
                    op=mybir.AluOpType.add,
                ).then_inc(sems["vord"], 1)
                v.wait_ge(sems["vord"], 2)
                v.tensor_scalar_mul(s_small[:], s_small[:], SCALING).then_inc(
                    sems["vord"], 1
                )
                v.wait_ge(sems["vord"], 3)
                v.tensor_copy(hi_small[:], s_small[:]).then_inc(
                    sems["vord"], 1
                )
                v.wait_ge(sems["vord"], 4)
                v.tensor_sub(lo_small[:], s_small[:], hi_small[:]).then_inc(
                    sems["dset"], 1
                )
                # shift diag (prod reuse is safe: the reduce above completed
                # before its consumer issued)
                v.wait_ge(sems["sdmaC"], 16)
                v.wait_ge(sems["sdmaD"], 16)
                v.tensor_mul(
                    prod[:].rearrange("p (c r) -> p c r", c=C),
                    a2_t[:].rearrange("p (c r) -> p c r", c=C),
                    b2_t[:].rearrange("p (r c) -> p c r", r=RANK),
                ).then_inc(sems["vord"], 1)
                v.wait_ge(sems["vord"], 5)
                v.tensor_reduce(
                    out=t_small[:].rearrange("p (c u) -> p c u", u=1),
                    in_=prod[:].rearrange("p (c r) -> p c r", c=C),
                    axis=mybir.AxisListType.X,
                    op=mybir.AluOpType.add,
                ).then_inc(sems["vord"], 1)
                v.wait_ge(sems["vord"], 6)
                v.tensor_scalar_mul(t_small[:], t_small[:], SCALING).then_inc(
                    sems["dset"], 1
                )

                def bn_pass(t):
                    # stats for tile t (runs one tile AHEAD of its stt
                    # pass, so the sqrt -> stdT-cast chain completes long
                    # before the PE needs stdT)
                    xt = xb[t % NXB]
                    for c in range(NBN):
                        if c % 4 == 0:
                            v.wait_ge(loadS[c // 4][t % NXB],
                                      16 * (t // NXB + 1))
                        bn = v.bn_stats(
                            out=stats[:].rearrange("p (c s) -> p c s", s=6)[
                                :, c, :
                            ],
                            in_=xt[:, c * BN_F:(c + 1) * BN_F],
                        )
                        if c == NBN - 1:
                            bn.then_inc(sems["bnend"], 1)
                    # own-sem wait: bn_stats writes committed before aggr
                    v.wait_ge(sems["bnend"], t + 1)
                    v.bn_aggr(
                        out=mv[t % 2][:],
                        in_=stats[:].rearrange("p (c s) -> p c s", s=6),
                    ).then_inc(sems["stats"], 1)

                bn_pass(0)
                for t in range(NTILES):
                    xt = xb[t % NXB]
                    if t + 1 < NTILES:
                        bn_pass(t + 1)
                    # own-sem wait: aggr's mv writes committed (stt reads mv)
                    v.wait_ge(sems["stats"], t + 1)
                    for c in range(NCHUNK):
                        g = NCHUNK * t + c
                        if t == 0:
                            # pz[c%NPZ] double-filled + scale_bc chunk c
                            v.wait_ge(sems["bset"], 5 + (c % NPZ))
                        if g >= NPZ:
                            # psum buffer g%NPZ free (ACT copied g-NPZ)
                            v.wait_ge(sems["copy"], g - NPZ + 1)
                        v.scalar_tensor_tensor(
                            out=pz[g % NPZ][:],
                            in0=xt[:, c * CHUNK:(c + 1) * CHUNK],
                            scalar=mv[t % 2][:, 0:1],
                            in1=scale_bc[:, c * CHUNK:(c + 1) * CHUNK],
                            op0=mybir.AluOpType.subtract,
                            op1=mybir.AluOpType.mult,
                        ).then_inc(sems["stt"], 1)
                        if c == 1:
                            # rstd for this tile, off the stt critical path:
                            # ACT's sqrt(t) fired right after aggr; the
                            # first copy only needs rstd ~2.5us from now
                            if t >= 4:
                                # rstd buffer free (copies of t-4 done)
                                v.wait_ge(sems["copy"], NCHUNK * (t - 3))
                            v.wait_ge(sems["std"], t + 1)
                            v.reciprocal(
                                rstdb[t % 4][:], stdb[t % 4][:]
                            ).then_inc(sems["rstd"], 1)

            @block.tensor
            def _(te):
                # scale broadcast: psum = ones2.T @ [hi;lo] = hi+lo, round r
                # fills pz[r%NPZ] with scale_bc chunk r (f32-accurate).
                # start=True also pre-sets PSUM has_written bits.
                te.wait_ge(sems["const"], 2)
                te.wait_ge(sems["gset"], 32)
                for i, r in enumerate(BC_ORDER):
                    if i >= NPZ:
                        # ACT copied round BC_ORDER[i-NPZ] out of pz[r%NPZ]
                        te.wait_ge(sems["bset"], i - NPZ + 1)
                    for s in range(NSL):
                        mm = nc.tensor.matmul(
                            pz[r % NPZ][:, s * 512:(s + 1) * 512],
                            ones2[:],
                            hilo_row[:, r * CHUNK + s * 512:
                                     r * CHUNK + (s + 1) * 512],
                            start=True,
                            stop=True,
                        )
                        if s == NSL - 1:
                            mm.then_inc(sems["pbc"], 1)
                te.wait_ge(sems["gset"], 48)  # sh_row resident
                for t in range(NTILES):
                    te.wait_ge(stdTS[t % 4], 16 * (t // 4 + 1))
                    for c in range(NCHUNK):
                        g = NCHUNK * t + c
                        te.wait_ge(sems["stt"], g + 1)
                        for s in range(NSL):
                            j = c * CHUNK + s * 512
                            mm = nc.tensor.matmul(
                                pz[g % NPZ][:, s * 512:(s + 1) * 512],
                                stdT[t % 4][:],
                                sh_row[:, j:j + 512],
                                start=False,
                                stop=True,
                                skip_group_check=True,
                            )
                            if s == NSL - 1:
                                mm.then_inc(sems["acc"], 1)

            @block.scalar
            def _(sc):
                sc.wait_ge(sems["const"], 1)  # eps
                # setup: copy broadcast rounds PSUM -> scale_bc (f32)
                for i, r in enumerate(BC_ORDER):
                    sc.wait_ge(sems["pbc"], i + 1)
                    sc.activation(
                        out=scale_bc[:, r * CHUNK:(r + 1) * CHUNK],
                        in_=pz[r % NPZ][:],
                        func=mybir.ActivationFunctionType.Copy,
                        bias=0.0,
                        scale=1.0,
                    ).then_inc(sems["bset"], 1)
                def store_q(tt, q):
                    # store quarter q of tile tt; callers place this where
                    # the copy-sem wait is already satisfied (no ACT stall)
                    sc.wait_ge(sems["copy"], NCHUNK * tt + 2 * (q + 1))
                    sc.dma_start(
                        out=y[tt * P:(tt + 1) * P, q * QTR:(q + 1) * QTR],
                        in_=outb[q][:],
                    ).then_inc(storeS[q], 16)

                for t in range(NTILES):
                    sc.wait_ge(sems["stats"], t + 1)
                    if t >= 4:
                        # std buffer free (gpsimd copied std of tile t-4)
                        sc.wait_ge(stdTS[t % 4], 16 * (t // 4))
                    sc.activation(
                        out=stdb[t % 4][:],
                        in_=mv[t % 2][:, 1:2],
                        func=mybir.ActivationFunctionType.Sqrt,
                        bias=eps_t[:],
                        scale=1.0,
                    ).then_inc(sems["std"], 1)
                    if t >= 1:
                        # tile t-1's last quarter: its copy-wait became
                        # free while bn_stats(t) ran on the DVE
                        store_q(t - 1, 3)
                    sc.wait_ge(sems["rstd"], t + 1)
                    for c in range(NCHUNK):
                        g = NCHUNK * t + c
                        q = c // 2
                        off = (c % 2) * CHUNK
                        sc.wait_ge(sems["acc"], g + 1)
                        if c % 2 == 0 and t >= 1:
                            # quarter buffer q free (store of tile t-1 done)
                            sc.wait_ge(storeS[q], 16 * t)
                        sc.activation(
                            out=outb[q][:, off:off + CHUNK],
                            in_=pz[g % NPZ][:],
                            func=mybir.ActivationFunctionType.Copy,
                            bias=0.0,
                            scale=rstdb[t % 4][:],
                        ).then_inc(sems["copy"], 1)
                        # stores lag their data by 2 copies so the copy-sem
                        # wait inside store_q is pre-satisfied.  Last tile:
                        # no later tile needs outb, so defer all its stores
                        # past the copies -- store dispatches between copies
                        # delay the PSUM recycle the final stt chunks wait on
                        if t < NTILES - 1:
                            if c == 3:
                                store_q(t, 0)
                            elif c == 5:
                                store_q(t, 1)
                            elif c == 7:
                                store_q(t, 2)
                for q in range(4):
                    store_q(NTILES - 1, q)

    return nc


def kernel(x, lora_scale_A, lora_scale_B, lora_shift_A, lora_shift_B):
    x = np.ascontiguousarray(np.asarray(x, dtype=np.float32).reshape(-1, N))
    args = {
        "lora_scale_A": np.ascontiguousarray(lora_scale_A, dtype=np.float32),
        "lora_scale_B": np.ascontiguousarray(lora_scale_B, dtype=np.float32),
        "lora_shift_A": np.ascontiguousarray(lora_shift_A, dtype=np.float32),
        "lora_shift_B": np.ascontiguousarray(lora_shift_B, dtype=np.float32),
    }
    in_maps = [
        {"x_shard": x[i * ROWS:(i + 1) * ROWS], **args} for i in range(NCORES)
    ]
    nc = build_nc()
    res = run_bass_kernel_spmd(nc, in_maps, core_ids=list(range(NCORES)))
    out = np.concatenate(
        [res.results[i]["y_shard"] for i in range(NCORES)], axis=0
    )
    return out.reshape(B_DIM, S_DIM, N)


if __name__ == "__main__":
    import reference

    inputs = {k: np.asarray(v) for k, v in reference.setup_inputs().items()}
    expected = np.asarray(reference.reference(**inputs))
    actual = kernel(**inputs)
    err = np.abs(actual - expected)
    denom = np.abs(expected).max()
    print("max abs err:", err.max(), "rel:", err.max() / denom)


# revision 25
# speedup vs baseline: 1.0824x; 1.0824x over previous
"""LoRA LayerNorm Trainium2 kernel (8-core data-parallel, raw Bass).

out = x_hat * scale + shift, where
  x_hat    = (x - mean) * rsqrt(var + eps)        (LayerNorm over last dim)
  scale[i] = sum_r A_s[i,r] * B_s[r,i] * 2.0      (low-rank diagonal)
  shift[i] = sum_r A_h[i,r] * B_h[r,i] * 2.0

Sharding: x [2,4096,8192] -> 8192 rows, 1024 rows per core. LoRA params
replicated; each core computes scale/shift redundantly on device.

Per-core algorithm (rows on partitions, 8 tiles of [128, 8192]):
  setup: scale/shift diagonals via strided loads + DVE mul/reduce.
         scale broadcast [128, N] f32 built on-chip: K=2 PE matmul
         (ones2 x [hi;lo]) into PSUM + ACT copy to SBUF, where hi/lo is
         a two-term bf16 split of scale (error ~2^-17, not 2^-8). The
         start=True matmuls double as the PSUM has_written pre-set.
  per tile (8 psum banks as 4 chunk buffers of [128,1024]):
    DVE : bn_stats/bn_aggr -> mean,var; psum = (x - mean) * scale_bc
    ACT : std = sqrt(var+eps); out_sbuf = psum * rstd  (PSUM->SBUF)
    PE  : psum += std (x) shift  (K=1 bf16 rank-1, start=False)
    SP  : x tile loads (HWDGE, 2 MiB halves, 3 tile buffers)
    ACT : output stores (HWDGE ring)
    POOL: tiny SBUF->SBUF cast DMA std [128,1] f32 -> stdT [1,128] bf16

Same-engine RAW hazards (e.g. bn_aggr writing mv then the next DVE op
reading it) are NOT interlocked by the engines: they must be ordered
by waiting on the producer's own completion semaphore (cheap) -- raw
drains cost ~1.7us each and are avoided.
"""

import numpy as np
from contextlib import ExitStack

import concourse.bass as bass
from concourse import mybir
from concourse.bass_utils import run_bass_kernel_spmd

F32 = mybir.dt.float32
BF16 = mybir.dt.bfloat16

# Problem geometry (hardcoded; see module docstring)
B_DIM, S_DIM, N = 2, 4096, 8192
RANK = 4
SCALING = 2.0  # alpha / rank = 8 / 4
EPS = 1e-5
NCORES = 8
ROWS = B_DIM * S_DIM // NCORES  # 1024 rows per core
P = 128
NTILES = ROWS // P              # 8
NXB = 3                         # x tile buffers
CHUNK = 1024                    # psum chunk (2 banks)
NCHUNK = N // CHUNK             # 8
NPZ = 4                         # psum chunk buffers (8 banks total)
NSL = CHUNK // 512              # matmul slices per chunk (2)
HALF = N // 2                   # load granularity
QTR = N // 4                    # store granularity (4 outb buffers)
BN_F = 512                      # bn_stats max free dim
NBN = N // BN_F                 # 16
C = N // P                      # 64
# broadcast round order: second-fill rounds first so tile-0 stt chunks
# can start as soon as their pz buffer has had both fills copied out
BC_ORDER = [4, 5, 6, 7, 0, 1, 2, 3]


def build_nc() -> bass.Bass:
    nc = bass.Bass()

    x = nc.declare_dram_parameter("x_shard", [ROWS, N], F32, isOutput=False)
    sa = nc.declare_dram_parameter("lora_scale_A", [N, RANK], F32, isOutput=False)
    sb = nc.declare_dram_parameter("lora_scale_B", [RANK, N], F32, isOutput=False)
    ha = nc.declare_dram_parameter("lora_shift_A", [N, RANK], F32, isOutput=False)
    hb = nc.declare_dram_parameter("lora_shift_B", [RANK, N], F32, isOutput=False)
    y = nc.declare_dram_parameter("y_shard", [ROWS, N], F32, isOutput=True)

    with ExitStack() as ctx:
        ec = ctx.enter_context
        # big tiles
        xb = [ec(nc.sbuf_tensor(f"xb{i}", [P, N], F32)) for i in range(NXB)]
        outb = [ec(nc.sbuf_tensor(f"outb{i}", [P, QTR], F32)) for i in range(4)]
        scale_bc = ec(nc.sbuf_tensor("scale_bc", [P, N], F32))
        hilo_row = ec(nc.sbuf_tensor("hilo_row", [2, N], BF16))
        sh_row = ec(nc.sbuf_tensor("sh_row", [1, N], BF16))
        ones2 = ec(nc.sbuf_tensor("ones2", [2, P], BF16))
        # setup scratch (scale pair and shift pair loaded in parallel)
        a_t = ec(nc.sbuf_tensor("a_t", [P, C * RANK], F32))   # [128, 256]
        b_t = ec(nc.sbuf_tensor("b_t", [P, RANK * C], F32))
        a2_t = ec(nc.sbuf_tensor("a2_t", [P, C * RANK], F32))
        b2_t = ec(nc.sbuf_tensor("b2_t", [P, RANK * C], F32))
        prod = ec(nc.sbuf_tensor("prod", [P, C * RANK], F32))
        s_small = ec(nc.sbuf_tensor("s_small", [P, C], F32))  # [128, 64]
        t_small = ec(nc.sbuf_tensor("t_small", [P, C], F32))
        hi_small = ec(nc.sbuf_tensor("hi_small", [P, C], BF16))
        lo_small = ec(nc.sbuf_tensor("lo_small", [P, C], F32))
        # per-tile stats
        stats = ec(nc.sbuf_tensor("stats", [P, NBN * 6], F32))
        mv = [ec(nc.sbuf_tensor(f"mv{i}", [P, 2], F32)) for i in range(2)]
        stdb = [ec(nc.sbuf_tensor(f"stdb{i}", [P, 1], F32)) for i in range(4)]
        rstdb = [ec(nc.sbuf_tensor(f"rstdb{i}", [P, 1], F32)) for i in range(4)]
        stdT = [ec(nc.sbuf_tensor(f"stdT{i}", [1, P], BF16)) for i in range(4)]
        eps_t = ec(nc.sbuf_tensor("eps_t", [P, 1], F32))
        # psum: 4 chunk buffers x 2 banks
        pz = [ec(nc.psum_tensor(f"pz{i}", [P, CHUNK], F32)) for i in range(NPZ)]

        sems = {}
        load_names = [f"ld{q}{b}" for q in range(4) for b in range(NXB)]
        for s in (*load_names,
                  "store0", "store1", "store2", "store3",
                  "stdT0", "stdT1", "stdT2", "stdT3", "stt", "stats", "std", "rstd", "acc",
                  "copy", "const", "sdmaA", "sdmaB", "sdmaC", "sdmaD",
                  "dset", "gset", "pbc", "bset",
                  "vord", "bnend"):
            sems[s] = ec(nc.semaphore(s))
        # one sem per load-quarter DMA: DMAs sharing a sem can
        # interleave their 16 per-engine incs, so a mid-threshold does
        # NOT imply the first DMA completed.
        loadS = [[sems[f"ld{q}{b}"] for b in range(NXB)] for q in range(4)]
        storeS = [sems[f"store{i}"] for i in range(4)]
        stdTS = [sems[f"stdT{i}"] for i in range(4)]

        with nc.Block() as block:

            @block.sync
            def _(sp):
                # LoRA params first: tiny, HWDGE (fast), unblocks the
                # diag -> broadcast chain ~10us earlier than SWDGE
                sp.dma_start(
                    out=a_t[:],
                    in_=sa[:, :].rearrange("(p c) r -> p (c r)", p=P),
                ).then_inc(sems["sdmaA"], 16)
                sp.dma_start(
                    out=b_t[:].rearrange("p (r c) -> p r c", r=RANK),
                    in_=sb[:, :].rearrange("r (p c) -> p r c", p=P),
                ).then_inc(sems["sdmaB"], 16)
                sp.dma_start(
                    out=a2_t[:],
                    in_=ha[:, :].rearrange("(p c) r -> p (c r)", p=P),
                ).then_inc(sems["sdmaC"], 16)
                sp.dma_start(
                    out=b2_t[:].rearrange("p (r c) -> p r c", r=RANK),
                    in_=hb[:, :].rearrange("r (p c) -> p r c", p=P),
                ).then_inc(sems["sdmaD"], 16)
                for t in range(NTILES):
                    for qf in range(4):
                        if t >= NXB:
                            # xb quarter free once DVE finished the stt
                            # chunks of tile t-NXB that read it
                            sp.wait_ge(
                                sems["stt"],
                                NCHUNK * (t - NXB) + 2 * (qf + 1),
                            )
                        sp.dma_start(
                            out=xb[t % NXB][:, qf * QTR:(qf + 1) * QTR],
                            in_=x[t * P:(t + 1) * P, qf * QTR:(qf + 1) * QTR],
                        ).then_inc(loadS[qf][t % NXB], 16)

            @block.gpsimd
            def _(gp):
                gp.wait_ge(sems["dset"], 1)
                # hi/lo bf16 split of scale -> [2, N] bf16 row pair
                gp.dma_start(out=hilo_row[0:1, :], in_=hi_small[:]).then_inc(
                    sems["gset"], 16
                )
                gp.dma_start(out=hilo_row[1:2, :], in_=lo_small[:]).then_inc(
                    sems["gset"], 16
                )
                gp.wait_ge(sems["dset"], 2)
                # shift diag [128,64] f32 -> [1,N] bf16 row (cast DMA)
                gp.dma_start(out=sh_row[:], in_=t_small[:]).then_inc(
                    sems["gset"], 16
                )
                # per-tile: std [128,1] f32 -> stdT [1,128] bf16 (cast)
                for t in range(NTILES):
                    gp.wait_ge(sems["std"], t + 1)
                    if t >= 4:
                        # PE done reading stdT[t%4] (accums of tile t-4)
                        gp.wait_ge(sems["acc"], NCHUNK * (t - 3))
                    gp.dma_start(
                        out=stdT[t % 4][:], in_=stdb[t % 4][:]
                    ).then_inc(stdTS[t % 4], 16)

            @block.vector
            def _(v):
                v.memset(eps_t[:], EPS).then_inc(sems["const"], 1)
                v.memset(ones2[:], 1.0).then_inc(sems["const"], 1)
                # scale diag + hi/lo split.  vord orders same-engine RAW
                # (an op's reads may start before the previous op's writes
                # commit; waiting on the producer's completion sem is the
                # cheap alternative to a full pipe drain).
                v.wait_ge(sems["sdmaA"], 16)
                v.wait_ge(sems["sdmaB"], 16)
                v.tensor_mul(
                    prod[:].rearrange("p (c r) -> p c r", c=C),
                    a_t[:].rearrange("p (c r) -> p c r", c=C),
                    b_t[:].rearrange("p (r c) -> p c r", r=RANK),
                ).then_inc(sems["vord"], 1)
                v.wait_ge(sems["vord"], 1)
                v.tensor_reduce(
                    out=s_small[:].rearrange("p (c u) -> p c u", u=1),
                    in_=prod[:].rearrange("p (c r) -> p c r", c=C),
                    axis=mybir.AxisListType.X,
                    op=mybir.AluOpType.add,
                ).then_inc(sems["vord"], 1)
                v.wait_ge(sems["vord"], 2)
                v.tensor_scalar_mul(s_small[:], s_small[:], SCALING).then_inc(
                    sems["vord"], 1
                )
                v.wait_ge(sems["vord"], 3)
                v.tensor_copy(hi_small[:], s_small[:]).then_inc(
                    sems["vord"], 1
                )
                v.wait_ge(sems["vord"], 4)
                v.tensor_sub(lo_small[:], s_small[:], hi_small[:]).then_inc(
                    sems["dset"], 1
                )
                # shift diag (prod reuse is safe: the reduce above completed
                # before its consumer issued)
                v.wait_ge(sems["sdmaC"], 16)
                v.wait_ge(sems["sdmaD"], 16)
                v.tensor_mul(
                    prod[:].rearrange("p (c r) -> p c r", c=C),
                    a2_t[:].rearrange("p (c r) -> p c r", c=C),
                    b2_t[:].rearrange("p (r c) -> p c r", r=RANK),
                ).then_inc(sems["vord"], 1)
                v.wait_ge(sems["vord"], 5)
                v.tensor_reduce(
                    out=t_small[:].rearrange("p (c u) -> p c u", u=1),
                    in_=prod[:].rearrange("p (c r) -> p c r", c=C),
                    axis=mybir.AxisListType.X,
                    op=mybir.AluOpType.add,
                ).then_inc(sems["vord"], 1)
                v.wait_ge(sems["vord"], 6)
                v.tensor_scalar_mul(t_small[:], t_small[:], SCALING).then_inc(
                    sems["dset"], 1
                )

                def bn_pass(t):
                    # stats for tile t (runs one tile AHEAD of its stt
                    # pass, so the sqrt -> stdT-cast chain completes long
                    # before the PE needs stdT)
                    xt = xb[t % NXB]
                    for c in range(NBN):
                        if c % 4 == 0:
                            v.wait_ge(loadS[c // 4][t % NXB],
                                      16 * (t // NXB + 1))
                        bn = v.bn_stats(
                            out=stats[:].rearrange("p (c s) -> p c s", s=6)[
                                :, c, :
                            ],
                            in_=xt[:, c * BN_F:(c + 1) * BN_F],
                        )
                        if c == NBN - 1:
                            bn.then_inc(sems["bnend"], 1)
                    # own-sem wait: bn_stats writes committed before aggr
                    v.wait_ge(sems["bnend"], t + 1)
                    v.bn_aggr(
                        out=mv[t % 2][:],
                        in_=stats[:].rearrange("p (c s) -> p c s", s=6),
                    ).then_inc(sems["stats"], 1)

                bn_pass(0)
                for t in range(NTILES):
                    xt = xb[t % NXB]
                    if t + 1 < NTILES:
                        bn_pass(t + 1)
                    # own-sem wait: aggr's mv writes committed (stt reads mv)
                    v.wait_ge(sems["stats"], t + 1)
                    for c in range(NCHUNK):
                        g = NCHUNK * t + c
                        if t == 0:
                            # pz[c%NPZ] double-filled + scale_bc chunk c
                            v.wait_ge(sems["bset"], 5 + (c % NPZ))
                        if g >= NPZ:
                            # psum buffer g%NPZ free (ACT copied g-NPZ)
                            v.wait_ge(sems["copy"], g - NPZ + 1)
                        v.scalar_tensor_tensor(
                            out=pz[g % NPZ][:],
                            in0=xt[:, c * CHUNK:(c + 1) * CHUNK],
                            scalar=mv[t % 2][:, 0:1],
                            in1=scale_bc[:, c * CHUNK:(c + 1) * CHUNK],
                            op0=mybir.AluOpType.subtract,
                            op1=mybir.AluOpType.mult,
                        ).then_inc(sems["stt"], 1)
                        if c == 1:
                            # rstd for this tile, off the stt critical path:
                            # ACT's sqrt(t) fired right after aggr; the
                            # first copy only needs rstd ~2.5us from now
                            if t >= 4:
                                # rstd buffer free (copies of t-4 done)
                                v.wait_ge(sems["copy"], NCHUNK * (t - 3))
                            v.wait_ge(sems["std"], t + 1)
                            v.reciprocal(
                                rstdb[t % 4][:], stdb[t % 4][:]
                            ).then_inc(sems["rstd"], 1)

            @block.tensor
            def _(te):
                # scale broadcast: psum = ones2.T @ [hi;lo] = hi+lo, round r
                # fills pz[r%NPZ] with scale_bc chunk r (f32-accurate).
                # start=True also pre-sets PSUM has_written bits.
                te.wait_ge(sems["const"], 2)
                te.wait_ge(sems["gset"], 32)
                for i, r in enumerate(BC_ORDER):
                    if i >= NPZ:
                        # ACT copied round BC_ORDER[i-NPZ] out of pz[r%NPZ]
                        te.wait_ge(sems["bset"], i - NPZ + 1)
                    for s in range(NSL):
                        mm = nc.tensor.matmul(
                            pz[r % NPZ][:, s * 512:(s + 1) * 512],
                            ones2[:],
                            hilo_row[:, r * CHUNK + s * 512:
                                     r * CHUNK + (s + 1) * 512],
                            start=True,
                            stop=True,
                        )
                        if s == NSL - 1:
                            mm.then_inc(sems["pbc"], 1)
                te.wait_ge(sems["gset"], 48)  # sh_row resident
                for t in range(NTILES):
                    te.wait_ge(stdTS[t % 4], 16 * (t // 4 + 1))
                    for c in range(NCHUNK):
                        g = NCHUNK * t + c
                        te.wait_ge(sems["stt"], g + 1)
                        for s in range(NSL):
                            j = c * CHUNK + s * 512
                            mm = nc.tensor.matmul(
                                pz[g % NPZ][:, s * 512:(s + 1) * 512],
                                stdT[t % 4][:],
                                sh_row[:, j:j + 512],
                                start=False,
                                stop=True,
                                skip_group_check=True,
                            )
                            if s == NSL - 1:
                                mm.then_inc(sems["acc"], 1)

            @block.scalar
            def _(sc):
                sc.wait_ge(sems["const"], 1)  # eps
                # setup: copy broadcast rounds PSUM -> scale_bc (f32)
                for i, r in enumerate(BC_ORDER):
                    sc.wait_ge(sems["pbc"], i + 1)
                    sc.activation(
                        out=scale_bc[:, r * CHUNK:(r + 1) * CHUNK],
                        in_=pz[r % NPZ][:],
                        func=mybir.ActivationFunctionType.Copy,
                        bias=0.0,
                        scale=1.0,
                    ).then_inc(sems["bset"], 1)
                def store_q(tt, q):
                    # store quarter q of tile tt; callers place this where
                    # the copy-sem wait is already satisfied (no ACT stall)
                    sc.wait_ge(sems["copy"], NCHUNK * tt + 2 * (q + 1))
                    sc.dma_start(
                        out=y[tt * P:(tt + 1) * P, q * QTR:(q + 1) * QTR],
                        in_=outb[q][:],
                    ).then_inc(storeS[q], 16)

                for t in range(NTILES):
                    sc.wait_ge(sems["stats"], t + 1)
                    if t >= 4:
                        # std buffer free (gpsimd copied std of tile t-4)
                        sc.wait_ge(stdTS[t % 4], 16 * (t // 4))
                    sc.activation(
                        out=stdb[t % 4][:],
                        in_=mv[t % 2][:, 1:2],
                        func=mybir.ActivationFunctionType.Sqrt,
                        bias=eps_t[:],
                        scale=1.0,
                    ).then_inc(sems["std"], 1)
                    if t >= 1:
                        # tile t-1's last quarter: its copy-wait became
                        # free while bn_stats(t) ran on the DVE
                        store_q(t - 1, 3)
                    sc.wait_ge(sems["rstd"], t + 1)
                    for c in range(NCHUNK):
                        g = NCHUNK * t + c
                        q = c // 2
                        off = (c % 2) * CHUNK
                        sc.wait_ge(sems["acc"], g + 1)
                        if c % 2 == 0 and t >= 1:
                            # quarter buffer q free (store of tile t-1 done)
                            sc.wait_ge(storeS[q], 16 * t)
                        sc.activation(
                            out=outb[q][:, off:off + CHUNK],
                            in_=pz[g % NPZ][:],
                            func=mybir.ActivationFunctionType.Copy,
                            bias=0.0,
                            scale=rstdb[t % 4][:],
                        ).then_inc(sems["copy"], 1)
                        # stores lag their data by 2 copies so the copy-sem
                        # wait inside store_q is pre-satisfied
                        if c == 3:
                            store_q(t, 0)
                        elif c == 5:
                            store_q(t, 1)
                        elif c == 7:
                            store_q(t, 2)
                store_q(NTILES - 1, 3)

    return nc


def kernel(x, lora_scale_A, lora_scale_B, lora_shift_A, lora_shift_B):
    x = np.ascontiguousarray(np.asarray(x, dtype=np.float32).reshape(-1, N))
    args = {
        "lora_scale_A": np.ascontiguousarray(lora_scale_A, dtype=np.float32),
        "lora_scale_B": np.ascontiguousarray(lora_scale_B, dtype=np.float32),
        "lora_shift_A": np.ascontiguousarray(lora_shift_A, dtype=np.float32),
        "lora_shift_B": np.ascontiguousarray(lora_shift_B, dtype=np.float32),
    }
    in_maps = [
        {"x_shard": x[i * ROWS:(i + 1) * ROWS], **args} for i in range(NCORES)
    ]
    nc = build_nc()
    res = run_bass_kernel_spmd(nc, in_maps, core_ids=list(range(NCORES)))
    out = np.concatenate(
        [res.results[i]["y_shard"] for i in range(NCORES)], axis=0
    )
    return out.reshape(B_DIM, S_DIM, N)


if __name__ == "__main__":
    import reference

    inputs = {k: np.asarray(v) for k, v in reference.setup_inputs().items()}
    expected = np.asarray(reference.reference(**inputs))
    actual = kernel(**inputs)
    err = np.abs(actual - expected)
    denom = np.abs(expected).max()
    print("max abs err:", err.max(), "rel:", err.max() / denom)
